# revision 3
# baseline (speedup 1.0000x reference)
import sys

import numpy as np

if "/opt/trn_rl_repo" not in sys.path:
    sys.path.insert(0, "/opt/trn_rl_repo")

import ml_dtypes

BF = ml_dtypes.bfloat16

B = 4
H = 128
F_OUT = 3
NBLK = 3
N_FULL = 10000
P_CORES = 8
F_IN = 963


def _tiles(total, t=128):
    out = []
    while total > 0:
        out.append(min(t, total))
        total -= t
    return out


def build_nc(N, P, F_in, group_size=5):
    """Build the SPMD Bass program (identical on all cores).

    Sharding: adj is row-sharded (NL=N/P rows per core), batches replicated.
    Per GCN layer each core computes xw=x@w for its rows, all-gathers the
    bf16 xw across cores, then computes adj_rows @ XW_full on the PE with
    adjT k-tiles as stationary operands. Residual state x stays row-major
    fp32 in SBUF. Biases are identically zero in this model and are skipped.
    """
    from concourse import bacc, bass, tile, mybir

    f32 = mybir.dt.float32
    bf16 = mybir.dt.bfloat16
    Relu = mybir.ActivationFunctionType.Relu
    Copy = mybir.ActivationFunctionType.Copy
    add = mybir.AluOpType.add

    NL = N // P
    MT = _tiles(NL)
    NMT = len(MT)
    m_offs = [sum(MT[:i]) for i in range(NMT)]
    KT = _tiles(N)
    NKT = len(KT)
    KF = (F_in + 127) // 128
    groups = [list(range(i, min(i + group_size, NMT))) for i in range(0, NMT, group_size)]
    out_chunks = []
    c0 = 0
    while c0 < NL:
        out_chunks.append((c0, min(c0 + 512, NL)))
        c0 += 512

    nc = bacc.Bacc(trn_type="TRN2", target_bir_lowering=False, num_devices=P)

    x0T = nc.dram_tensor("x0T", [B, KF, 128, NL], bf16, kind="ExternalInput")
    adjT = nc.dram_tensor("adjT", [NKT, 128, NL], bf16, kind="ExternalInput")
    w_in = nc.dram_tensor("w_in", [KF, 128, 2 * H], bf16, kind="ExternalInput")
    w_res = nc.dram_tensor("w_res", [2 * NBLK, 128, 2 * H], bf16, kind="ExternalInput")
    w_out = nc.dram_tensor("w_out", [128, 2 * F_OUT], bf16, kind="ExternalInput")
    ident = nc.dram_tensor("ident", [128, 128], bf16, kind="ExternalInput")
    x_res_out = nc.dram_tensor("x_res_out", [B, NL, H], f32, kind="ExternalOutput")
    x_out_t = nc.dram_tensor("x_out_t", [B * F_OUT, NL], f32, kind="ExternalOutput")

    replica_groups = [list(range(P))]

    with tile.TileContext(nc) as tc:
        with (
            tc.tile_pool(name="wp", bufs=1) as wp,
            tc.tile_pool(name="sbp", bufs=1) as sbp,
            tc.tile_pool(name="stp", bufs=4) as stp,
            tc.tile_pool(name="pp", bufs=1, space="PSUM") as pp,
            tc.tile_pool(name="dp", bufs=2, space="DRAM") as dp,
        ):
            w_in_t = []
            for kf in range(KF):
                wt = wp.tile([128, 2 * H], bf16, tag=f"w_in{kf}", name=f"w_in{kf}")
                nc.sync.dma_start(wt[:, :], w_in[kf, :, :])
                w_in_t.append(wt)
            w_res_t = []
            for i in range(2 * NBLK):
                wt = wp.tile([128, 2 * H], bf16, tag=f"w_res{i}", name=f"w_res{i}")
                nc.sync.dma_start(wt[:, :], w_res[i, :, :])
                w_res_t.append(wt)
            w_out_t = wp.tile([128, 2 * F_OUT], bf16, tag="w_out", name="w_out")
            nc.sync.dma_start(w_out_t[:, :], w_out[:, :])
            ident_t = wp.tile([128, 128], bf16, tag="ident", name="ident")
            nc.sync.dma_start(ident_t[:, :], ident[:, :])
            xlwoT_sb = sbp.tile([F_OUT * B, NL], f32, tag="xlwoT", name="xlwoT_sb")

            xT_cur = {}
            xlw_cur = {}
            xres_cur = {}
            state = {"bounce_next": None}

            def new_bounce(cols):
                bb = dp.tile([NL, cols], bf16, tag="bounce", name="bounce")
                state["bounce_next"] = bb
                return bb

            def new_gath(cols):
                return dp.tile([N, cols], bf16, tag="gath", addr_space="Shared", name="gath")

            def emit_ag(bounce, gath):
                nc.gpsimd.collective_compute(
                    "AllGather",
                    mybir.AluOpType.bypass,
                    replica_groups=replica_groups,
                    ins=[bounce.opt()],
                    outs=[gath.opt()],
                )

            def emit_feature_h(g_next, mt):
                m, moff = MT[mt], m_offs[mt]
                bounce = state["bounce_next"]
                wt = w_res_t[g_next - 1]
                xlw_new = sbp.tile([128, B * H], f32, tag=f"xlw{mt}", bufs=2, name=f"xlw{mt}")
                for b in range(B):
                    pf = pp.tile([128, 2 * H], f32, tag="pfeat", bufs=2, name="pf")
                    nc.tensor.matmul(pf[0:m, :], xT_cur[(b, mt)][:, 0:m], wt[:, :])
                    xws = stp.tile([128, H], bf16, tag="xwst", name="xws")
                    nc.scalar.activation(xws[0:m, :], pf[0:m, 0:H], Copy)
                    nc.gpsimd.dma_start(bounce[moff : moff + m, b * H : (b + 1) * H], xws[0:m, :])
                    nc.vector.tensor_copy(xlw_new[0:m, b * H : (b + 1) * H], pf[0:m, H : 2 * H])
                xlw_cur[mt] = xlw_new

            def emit_out_feature(mt):
                m, moff = MT[mt], m_offs[mt]
                bounce = state["bounce_next"]
                xwo_st = sbp.tile([128, 16], bf16, tag="xwost", bufs=2, name="xwo_st")
                xlwo_st = sbp.tile([128, 16], bf16, tag="xlwost", bufs=2, name="xlwo_st")
                for b in range(B):
                    pf = pp.tile([128, 2 * F_OUT], f32, tag="pfeat", bufs=2, name="pfo")
                    nc.tensor.matmul(pf[0:m, :], xT_cur[(b, mt)][:, 0:m], w_out_t[:, :])
                    nc.scalar.activation(xwo_st[0:m, b * F_OUT : (b + 1) * F_OUT], pf[0:m, 0:F_OUT], Copy)
                    nc.scalar.activation(xlwo_st[0:m, b * F_OUT : (b + 1) * F_OUT], pf[0:m, F_OUT : 2 * F_OUT], Copy)
                nc.gpsimd.dma_start(bounce[moff : moff + m, 0 : B * F_OUT], xwo_st[0:m, 0 : B * F_OUT])
                pt = pp.tile([128, 128], bf16, tag="ptr", name="ptlo")
                nc.tensor.transpose(pt[0 : B * F_OUT, 0:m], xlwo_st[0:m, 0 : B * F_OUT], ident_t[0:m, 0:m])
                nc.scalar.activation(xlwoT_sb[:, moff : moff + m], pt[0 : B * F_OUT, 0:m], Copy)

            def emit_epilogue(g, mt, pacc):
                m, moff = MT[mt], m_offs[mt]
                s = sbp.tile([128, B * H], f32, tag="stmp", bufs=2, name="s")
                nc.vector.tensor_tensor(s[0:m, :], pacc[0:m, :], xlw_cur[mt][0:m, :], op=add)
                if g == 0:
                    xr = sbp.tile([128, B * H], f32, tag=f"xres{mt}", bufs=2, name=f"xres{mt}")
                    nc.scalar.activation(xr[0:m, :], s[0:m, :], Relu)
                    xres_cur[mt] = xr
                    yrow = sbp.tile([128, B * H], bf16, tag="yrow", bufs=2, name="yrow")
                    nc.scalar.activation(yrow[0:m, :], xr[0:m, :], Copy)
                elif g % 2 == 1:  # h1 of a res block
                    yrow = sbp.tile([128, B * H], bf16, tag="yrow", bufs=2, name="yrow")
                    nc.scalar.activation(yrow[0:m, :], s[0:m, :], Relu)
                else:  # h2: x = (x + relu(s)) * 0.5
                    h2f = sbp.tile([128, B * H], f32, tag="h2f", bufs=2, name="h2f")
                    nc.scalar.activation(h2f[0:m, :], s[0:m, :], Relu)
                    u = sbp.tile([128, B * H], f32, tag="stmp", bufs=2, name="u")
                    nc.vector.tensor_tensor(u[0:m, :], h2f[0:m, :], xres_cur[mt][0:m, :], op=add)
                    xr = sbp.tile([128, B * H], f32, tag=f"xres{mt}", bufs=2, name=f"xres{mt}")
                    nc.scalar.activation(xr[0:m, :], u[0:m, :], Copy, scale=0.5)
                    xres_cur[mt] = xr
                    yrow = sbp.tile([128, B * H], bf16, tag="yrow", bufs=2, name="yrow")
                    nc.scalar.activation(yrow[0:m, :], xr[0:m, :], Copy)
                for b in range(B):
                    pt = pp.tile([128, 128], bf16, tag="ptr", name="pt")
                    nc.tensor.transpose(pt[:, 0:m], yrow[0:m, b * H : (b + 1) * H], ident_t[0:m, 0:m])
                    xt = sbp.tile([128, 128], bf16, tag=f"xT_{b}_{mt}", bufs=2, name=f"xT_{b}_{mt}")
                    nc.scalar.activation(xt[:, 0:m], pt[:, 0:m], Copy)
                    xT_cur[(b, mt)] = xt
                if g < 6:
                    emit_feature_h(g + 1, mt)
                else:
                    emit_out_feature(mt)
                    for b in range(B):
                        nc.sync.dma_start(
                            x_res_out[b, moff : moff + m, :],
                            xres_cur[mt][0:m, b * H : (b + 1) * H],
                        )

            def emit_adj_stage(g, gath):
                for grp in groups:
                    c0 = m_offs[grp[0]]
                    c1 = m_offs[grp[-1]] + MT[grp[-1]]
                    paccs = {}
                    for mi, mt in enumerate(grp):
                        paccs[mt] = pp.tile([128, B * H], f32, tag=f"pacc{mi}", name=f"pacc{mi}")
                    for k in range(NKT):
                        kp = KT[k]
                        xwk = stp.tile([128, B * H], bf16, tag="xwk", name="xwk")
                        nc.sync.dma_start(xwk[0:kp, :], gath[k * 128 : k * 128 + kp, :])
                        at = stp.tile([128, NL], bf16, tag="adjt", name="at")
                        nc.scalar.dma_start(at[0:kp, 0 : c1 - c0], adjT[k, 0:kp, c0:c1])
                        for mt in grp:
                            o0 = m_offs[mt] - c0
                            m = MT[mt]
                            nc.tensor.matmul(
                                paccs[mt][0:m, :],
                                at[0:kp, o0 : o0 + m],
                                xwk[0:kp, :],
                                start=(k == 0),
                                stop=(k == NKT - 1),
                            )
                    for mt in grp:
                        emit_epilogue(g, mt, paccs[mt])

            def emit_out_adj_stage(gath):
                pouts = []
                for ci, (c0, c1) in enumerate(out_chunks):
                    pouts.append(pp.tile([B * F_OUT, 512], f32, tag=f"pacc{ci}", name=f"pout{ci}"))
                for k in range(NKT):
                    kp = KT[k]
                    xwok = stp.tile([128, 16], bf16, tag="xwok", name="xwok")
                    nc.sync.dma_start(xwok[0:kp, :], gath[k * 128 : k * 128 + kp, :])
                    at = stp.tile([128, NL], bf16, tag="adjt", name="ato")
                    nc.scalar.dma_start(at[0:kp, :], adjT[k, 0:kp, :])
                    for ci, (c0, c1) in enumerate(out_chunks):
                        nc.tensor.matmul(
                            pouts[ci][:, 0 : c1 - c0],
                            xwok[0:kp, 0 : B * F_OUT],
                            at[0:kp, c0:c1],
                            start=(k == 0),
                            stop=(k == NKT - 1),
                        )
                youtT = sbp.tile([B * F_OUT, NL], f32, tag="youtT", name="youtT")
                for ci, (c0, c1) in enumerate(out_chunks):
                    nc.vector.tensor_tensor(
                        youtT[:, c0:c1], pouts[ci][:, 0 : c1 - c0], xlwoT_sb[:, c0:c1], op=add
                    )
                nc.sync.dma_start(x_out_t[:, :], youtT[:, :])

            # ---- layer 0 feature stage (from DRAM x0T) ----
            bounce0 = new_bounce(B * H)
            for b in range(B):
                x0t_t = []
                for kf in range(KF):
                    xt0 = stp.tile([128, NL], bf16, tag=f"x0t{kf}", bufs=2, name=f"x0t{kf}")
                    nc.sync.dma_start(xt0[:, :], x0T[b, kf, :, :])
                    x0t_t.append(xt0)
                for mt in range(NMT):
                    m, moff = MT[mt], m_offs[mt]
                    if b == 0:
                        xlw_cur[mt] = sbp.tile(
                            [128, B * H], f32, tag=f"xlw{mt}", bufs=2, name=f"xlw{mt}"
                        )
                    pf = pp.tile([128, 2 * H], f32, tag="pfeat", bufs=2, name="pf0")
                    for kf in range(KF):
                        nc.tensor.matmul(
                            pf[0:m, :],
                            x0t_t[kf][:, moff : moff + m],
                            w_in_t[kf][:, :],
                            start=(kf == 0),
                            stop=(kf == KF - 1),
                        )
                    xws = stp.tile([128, H], bf16, tag="xwst", name="xws0")
                    nc.scalar.activation(xws[0:m, :], pf[0:m, 0:H], Copy)
                    nc.gpsimd.dma_start(bounce0[moff : moff + m, b * H : (b + 1) * H], xws[0:m, :])
                    nc.vector.tensor_copy(xlw_cur[mt][0:m, b * H : (b + 1) * H], pf[0:m, H : 2 * H])

            gath_g = new_gath(B * H)
            emit_ag(bounce0, gath_g)

            # ---- gcn layers 0..6 adj stages (epilogues emit next feature stage) ----
            for g in range(7):
                cols = B * H if g < 6 else 16
                new_bounce(cols)
                emit_adj_stage(g, gath_g)
                gath_next = new_gath(cols)
                emit_ag(state["bounce_next"], gath_next)
                gath_g = gath_next

            emit_out_adj_stage(gath_g)

    nc.finalize()
    return nc


def _host_prep(shape_verts, adj, in_w, in_lw, res_w1, res_lw1, res_w2, res_lw2, out_w, out_lw, N, P, F_in):
    NL = N // P
    KT = _tiles(N)
    NKT = len(KT)
    KF = (F_in + 127) // 128
    FP = KF * 128

    adjT_full = np.zeros((NKT * 128, N), dtype=BF)
    adjT_full[:N, :] = adj.T.astype(BF)
    adjT_full = adjT_full.reshape(NKT, 128, N)

    w_cat = np.zeros((FP, 2 * H), np.float32)
    w_cat[:F_in, :H] = in_w
    w_cat[:F_in, H:] = in_lw
    w_in_h = w_cat.reshape(KF, 128, 2 * H).astype(BF)

    w_res_h = np.zeros((2 * NBLK, H, 2 * H), np.float32)
    for i in range(NBLK):
        w_res_h[2 * i, :, :H] = res_w1[i]
        w_res_h[2 * i, :, H:] = res_lw1[i]
        w_res_h[2 * i + 1, :, :H] = res_w2[i]
        w_res_h[2 * i + 1, :, H:] = res_lw2[i]
    w_res_h = w_res_h.astype(BF)

    w_out_h = np.concatenate([out_w, out_lw], axis=1).astype(BF)
    ident_h = np.eye(128, dtype=np.float32).astype(BF)

    in_maps = []
    for c in range(P):
        rows = slice(c * NL, (c + 1) * NL)
        svc = shape_verts[:, rows, :].transpose(0, 2, 1).astype(BF)  # [B, F_in, NL]
        x0T_c = np.zeros((B, KF * 128, NL), dtype=BF)
        x0T_c[:, :F_in, :] = svc
        x0T_c = x0T_c.reshape(B, KF, 128, NL)
        adjT_c = np.ascontiguousarray(adjT_full[:, :, rows])
        in_maps.append(
            {
                "x0T": x0T_c,
                "adjT": adjT_c,
                "w_in": w_in_h,
                "w_res": w_res_h,
                "w_out": w_out_h,
                "ident": ident_h,
            }
        )
    return in_maps


def run(inputs, N, P, F_in, trace=False):
    from concourse import bass_utils

    nc = build_nc(N, P, F_in)
    in_maps = _host_prep(
        inputs["shape_verts"], inputs["adj"],
        inputs["in_w"], inputs["in_lw"],
        inputs["res_w1"], inputs["res_lw1"],
        inputs["res_w2"], inputs["res_lw2"],
        inputs["out_w"], inputs["out_lw"],
        N, P, F_in,
    )
    res = bass_utils.run_bass_kernel_spmd(nc, in_maps, list(range(P)), trace=trace)
    NL = N // P
    x_full = np.empty((B, N, H), np.float32)
    x_out = np.empty((B, N, F_OUT), np.float32)
    for c in range(P):
        x_full[:, c * NL : (c + 1) * NL, :] = res.results[c]["x_res_out"]
        yt = np.asarray(res.results[c]["x_out_t"]).reshape(B, F_OUT, NL)
        x_out[:, c * NL : (c + 1) * NL, :] = yt.transpose(0, 2, 1)
    return (x_out, x_full), res


def kernel(**inputs):
    (x_out, x_full), _ = run(inputs, N_FULL, P_CORES, F_IN)
    return (x_out, x_full)
